# revision 1
# baseline (speedup 1.0000x reference)
"""Trainium2 Bass kernel for AttentionFuserV2 (dense transformer, 2 attention
blocks + mean-pool), data-parallel over 8 NeuronCores.

Math per batch element (x = [L, D] slice of embeddings):
  block(x, Win, Wout, tanh?):
    q      = x @ Win^T                # [L, D]
    s      = q @ x^T                  # [L, L]
    w      = softmax(s, axis=-1)
    mix    = w @ x                    # [L, D]
    out    = [mix, q] @ Wout^T        # [L, D]
    return tanh(out) if tanh? else out
  x1 = block(x,  Win1, Wout1, tanh=True)
  x2 = block(x1, Win2, Wout2, tanh=False)
  pooled = mean_l(x2)                 # [D]

Block-2 shortcut (linearity of mean): pooled = (1/L) * Wout2 @ combbar where
  combbar = [ mixbar ; qbar ],
  qbar    = sum_l q2[l, :],
  mixbar  = sum_d' c2[d'] * x1[d', :], c2[m] = sum_l softmax(s2)[l, m]
so mix2/out2 are never materialized ([L,D]+[L,2D] matmuls skipped).

Layouts: everything partition-inner chunked: logical index i <-> (k=i//128,
p=i%128); a tile [128, K, F] holds a [128*K, F] matrix. Matmuls are
out[M,N] = lhsT[K,M].T @ rhs[K,N] with K=128 contraction chunks accumulated
in PSUM. Transposes are done with HWDGE DMA-transpose on bf16 (no PE cycles).
Softmax skips the max subtraction (scores are bounded ~ +-65, safe in fp32
exp and bf16 storage) and uses the ACT exp's accum_out for the row sums.
"""

import os
import sys

import numpy as np

if "/opt/trn_rl_repo" not in sys.path:
    sys.path.insert(0, "/opt/trn_rl_repo")

P = 128
L = 512          # sequence length
D = 512          # model dim
C = 2 * D        # concat dim
KL = L // P      # 4
KD = D // P      # 4
KC = C // P      # 8
N_FULL = 256     # full batch
N_CORES = 8
B = N_FULL // N_CORES  # 32 batch elements per core

_NC_CACHE = {}


def build_nc(b_per_core: int):
    import concourse.bass as bass  # noqa: F401
    import concourse.mybir as mybir
    from concourse import bacc
    from concourse.tile import TileContext

    FP32 = mybir.dt.float32
    BF16 = mybir.dt.bfloat16
    EXP = mybir.ActivationFunctionType.Exp
    TANH = mybir.ActivationFunctionType.Tanh
    COPY = mybir.ActivationFunctionType.Copy
    AX = mybir.AxisListType.X

    nc = bacc.Bacc()

    emb = nc.declare_dram_parameter("embeddings", [b_per_core, L, D], FP32, isOutput=False)
    w_in1 = nc.declare_dram_parameter("W_in1", [D, D], FP32, isOutput=False)
    w_out1 = nc.declare_dram_parameter("W_out1", [D, C], FP32, isOutput=False)
    w_in2 = nc.declare_dram_parameter("W_in2", [D, D], FP32, isOutput=False)
    w_out2 = nc.declare_dram_parameter("W_out2", [D, C], FP32, isOutput=False)
    out = nc.declare_dram_parameter("out", [b_per_core, D], FP32, isOutput=True)

    with TileContext(nc) as tc:
        import contextlib

        ctx = contextlib.ExitStack()
        with ctx:
            wpool = ctx.enter_context(tc.tile_pool(name="wpool", bufs=1))
            xpool = ctx.enter_context(tc.tile_pool(name="xpool", bufs=2))
            apool = ctx.enter_context(tc.tile_pool(name="apool", bufs=2))
            spool = ctx.enter_context(tc.tile_pool(name="spool", bufs=3))
            psum = ctx.enter_context(tc.tile_pool(name="psum", bufs=4, space="PSUM"))

            # ---- one-time weight prep: f32 HBM -> bf16 transposed SBUF ----
            # WinT[i][p, kd, e]  = Win_i[e, kd*128+p]   (Win^T, d on partitions)
            # WoutT[i][p, kc, d] = Wout_i[d, kc*128+p]  (Wout^T, c on partitions)
            win_T = []
            wout_T = []
            with tc.tile_pool(name="wprep", bufs=2) as prep:
                for i, w_in in enumerate((w_in1, w_in2)):
                    wtmp = prep.tile([P, KD, D], FP32, name=f"wtmp_in{i}", tag="wtmp")
                    nc.gpsimd.dma_start(out=wtmp, in_=w_in.rearrange("(k p) d -> p k d", p=P))
                    wtmp_bf = prep.tile([P, KD, D], BF16, name=f"wtmpb_in{i}", tag="wtmpb")
                    nc.vector.tensor_copy(out=wtmp_bf, in_=wtmp)
                    wT = wpool.tile([P, KD, D], BF16, name=f"win_T{i}")
                    for ke in range(KD):
                        nc.sync.dma_start_transpose(
                            wT[:, :, ke * P : (ke + 1) * P], wtmp_bf[:, ke, :]
                        )
                    win_T.append(wT)
                for i, w_out in enumerate((w_out1, w_out2)):
                    wtmp2 = prep.tile([P, KD, C], FP32, name=f"wtmp_out{i}", tag="wtmp2")
                    nc.gpsimd.dma_start(out=wtmp2, in_=w_out.rearrange("(k p) c -> p k c", p=P))
                    wtmp2_bf = prep.tile([P, KD, C], BF16, name=f"wtmpb_out{i}", tag="wtmp2b")
                    nc.vector.tensor_copy(out=wtmp2_bf, in_=wtmp2)
                    wT = wpool.tile([P, KC, D], BF16, name=f"wout_T{i}")
                    for kd in range(KD):
                        nc.sync.dma_start_transpose(
                            wT[:, :, kd * P : (kd + 1) * P], wtmp2_bf[:, kd, :]
                        )
                    wout_T.append(wT)

            inv_L = 1.0 / float(L)

            for b in range(b_per_core):
                # ---- load x, cast bf16, build x^T ----
                x_f32 = xpool.tile([P, KL, D], FP32, name="x_f32")
                nc.gpsimd.dma_start(out=x_f32, in_=emb[b].rearrange("(k p) d -> p k d", p=P))
                x_nat = apool.tile([P, KL, D], BF16, name="x_nat")  # [p,k,d]=x[k*128+p,d]
                nc.gpsimd.tensor_copy(out=x_nat, in_=x_f32)
                xT = apool.tile([P, KD, L], BF16, name="xT")  # [p,k,m]=x[m,k*128+p]
                for km in range(KL):
                    nc.sync.dma_start_transpose(xT[:, :, km * P : (km + 1) * P], x_nat[:, km, :])

                # ---- block 1 ----
                # q1^T[e, l]
                q1T = apool.tile([P, KD, L], BF16, name="q1T")
                for eh in range(KD):
                    ps_q = psum.tile([P, L], FP32, name="ps_q", tag="mm")
                    for kd in range(KD):
                        nc.tensor.matmul(
                            ps_q,
                            lhsT=win_T[0][:, kd, eh * P : (eh + 1) * P],
                            rhs=xT[:, kd, :],
                            start=(kd == 0),
                            stop=(kd == KD - 1),
                        )
                    nc.vector.tensor_copy(out=q1T[:, eh, :], in_=ps_q)

                # scores1 -> exp (+row sums) -> normalized w1, all [l, m] natural
                w1 = apool.tile([P, KL, L], BF16, name="w1")
                for lh in range(KL):
                    ps_s = psum.tile([P, L], FP32, name="ps_s", tag="mm")
                    for ke in range(KD):
                        nc.tensor.matmul(
                            ps_s,
                            lhsT=q1T[:, ke, lh * P : (lh + 1) * P],
                            rhs=xT[:, ke, :],
                            start=(ke == 0),
                            stop=(ke == KD - 1),
                        )
                    rs = spool.tile([P, 1], FP32, name="rs", tag="rs")
                    nc.scalar.activation(out=w1[:, lh, :], in_=ps_s, func=EXP, accum_out=rs)
                    r = spool.tile([P, 1], FP32, name="r", tag="r")
                    nc.vector.reciprocal(out=r, in_=rs)
                    nc.vector.tensor_scalar_mul(w1[:, lh, :], w1[:, lh, :], r)

                # w1^T[m, l]
                w1T = apool.tile([P, KL, L], BF16, name="w1T")
                for lh in range(KL):
                    nc.sync.dma_start_transpose(w1T[:, :, lh * P : (lh + 1) * P], w1[:, lh, :])

                # mix1^T[d, l]
                mixT = apool.tile([P, KD, L], BF16, name="mixT")
                for dh in range(KD):
                    ps_m = psum.tile([P, L], FP32, name="ps_m", tag="mm")
                    for km in range(KL):
                        nc.tensor.matmul(
                            ps_m,
                            lhsT=x_nat[:, km, dh * P : (dh + 1) * P],
                            rhs=w1T[:, km, :],
                            start=(km == 0),
                            stop=(km == KL - 1),
                        )
                    nc.vector.tensor_copy(out=mixT[:, dh, :], in_=ps_m)

                # out1 = [mix1, q1] @ Wout1^T, tanh -> x1 (natural rows)
                x1_nat = apool.tile([P, KL, D], BF16, name="x1_nat")
                for lh in range(KL):
                    ps_o = psum.tile([P, D], FP32, name="ps_o", tag="mm")
                    for kc in range(KC):
                        lhsT_src = (
                            mixT[:, kc, lh * P : (lh + 1) * P]
                            if kc < KD
                            else q1T[:, kc - KD, lh * P : (lh + 1) * P]
                        )
                        nc.tensor.matmul(
                            ps_o,
                            lhsT=lhsT_src,
                            rhs=wout_T[0][:, kc, :],
                            start=(kc == 0),
                            stop=(kc == KC - 1),
                        )
                    nc.scalar.activation(out=x1_nat[:, lh, :], in_=ps_o, func=TANH)

                # ---- block 2 (pooled shortcut) ----
                x1T = apool.tile([P, KD, L], BF16, name="x1T")
                for km in range(KL):
                    nc.sync.dma_start_transpose(x1T[:, :, km * P : (km + 1) * P], x1_nat[:, km, :])

                q2T = apool.tile([P, KD, L], BF16, name="q2T")
                for eh in range(KD):
                    ps_q2 = psum.tile([P, L], FP32, name="ps_q2", tag="mm")
                    for kd in range(KD):
                        nc.tensor.matmul(
                            ps_q2,
                            lhsT=win_T[1][:, kd, eh * P : (eh + 1) * P],
                            rhs=x1T[:, kd, :],
                            start=(kd == 0),
                            stop=(kd == KD - 1),
                        )
                    nc.vector.tensor_copy(out=q2T[:, eh, :], in_=ps_q2)

                # scores2 -> e2 (unnormalized) + per-row 1/sum as bf16 columns
                e2 = apool.tile([P, KL, L], BF16, name="e2")
                r2b = spool.tile([P, KL], BF16, name="r2b", tag="r2b")
                for lh in range(KL):
                    ps_s2 = psum.tile([P, L], FP32, name="ps_s2", tag="mm")
                    for ke in range(KD):
                        nc.tensor.matmul(
                            ps_s2,
                            lhsT=q2T[:, ke, lh * P : (lh + 1) * P],
                            rhs=x1T[:, ke, :],
                            start=(ke == 0),
                            stop=(ke == KD - 1),
                        )
                    rs2 = spool.tile([P, 1], FP32, name="rs2", tag="rs")
                    nc.scalar.activation(out=e2[:, lh, :], in_=ps_s2, func=EXP, accum_out=rs2)
                    r2 = spool.tile([P, 1], FP32, name="r2", tag="r")
                    nc.vector.reciprocal(out=r2, in_=rs2)
                    nc.vector.tensor_copy(out=r2b[:, lh : lh + 1], in_=r2)

                # c2[m] = sum_l w2[l, m] : 16 tiny matmuls, column-sliced PSUM
                ps_c2 = psum.tile([P, KL], FP32, name="ps_c2", tag="small", bufs=2)
                for mh in range(KL):
                    for kl in range(KL):
                        nc.tensor.matmul(
                            ps_c2[:, mh : mh + 1],
                            lhsT=e2[:, kl, mh * P : (mh + 1) * P],
                            rhs=r2b[:, kl : kl + 1],
                            start=(kl == 0),
                            stop=(kl == KL - 1),
                        )
                c2b = spool.tile([P, KL], BF16, name="c2b", tag="c2b")
                nc.vector.tensor_copy(out=c2b, in_=ps_c2)

                # mixbar[d] = sum_m c2[m] * x1[m, d]
                ps_mb = psum.tile([P, KD], FP32, name="ps_mb", tag="small", bufs=2)
                for dh in range(KD):
                    for km in range(KL):
                        nc.tensor.matmul(
                            ps_mb[:, dh : dh + 1],
                            lhsT=x1_nat[:, km, dh * P : (dh + 1) * P],
                            rhs=c2b[:, km : km + 1],
                            start=(km == 0),
                            stop=(km == KL - 1),
                        )

                # qbar[e] = sum_l q2[l, e]
                qb_f32 = spool.tile([P, KD], FP32, name="qb_f32", tag="qb")
                for ke in range(KD):
                    nc.vector.reduce_sum(out=qb_f32[:, ke : ke + 1], in_=q2T[:, ke, :], axis=AX)

                # combbar (bf16 columns) = [mixbar ; qbar]
                cb = spool.tile([P, KC], BF16, name="cb", tag="cb")
                nc.vector.tensor_copy(out=cb[:, :KD], in_=ps_mb)
                nc.vector.tensor_copy(out=cb[:, KD:], in_=qb_f32)

                # pooled row = (1/L) * combbar @ Wout2^T
                ps_p = psum.tile([1, D], FP32, name="ps_p", tag="row", bufs=2)
                for kc in range(KC):
                    nc.tensor.matmul(
                        ps_p,
                        lhsT=cb[:, kc : kc + 1],
                        rhs=wout_T[1][:, kc, :],
                        start=(kc == 0),
                        stop=(kc == KC - 1),
                    )
                out_row = spool.tile([1, D], FP32, name="out_row", tag="out_row")
                nc.scalar.activation(out=out_row, in_=ps_p, func=COPY, scale=inv_L)
                nc.gpsimd.dma_start(out=out[b : b + 1, :], in_=out_row)

    nc.finalize()
    return nc


def _get_nc(b_per_core: int):
    if b_per_core not in _NC_CACHE:
        _NC_CACHE[b_per_core] = build_nc(b_per_core)
    return _NC_CACHE[b_per_core]


def kernel(embeddings, W_in1, W_out1, W_in2, W_out2):
    from concourse.bass_utils import run_bass_kernel_spmd

    embeddings = np.ascontiguousarray(np.asarray(embeddings, dtype=np.float32))
    W_in1 = np.ascontiguousarray(np.asarray(W_in1, dtype=np.float32))
    W_out1 = np.ascontiguousarray(np.asarray(W_out1, dtype=np.float32))
    W_in2 = np.ascontiguousarray(np.asarray(W_in2, dtype=np.float32))
    W_out2 = np.ascontiguousarray(np.asarray(W_out2, dtype=np.float32))

    assert embeddings.shape == (N_FULL, L, D), embeddings.shape

    nc = _get_nc(B)
    in_maps = [
        {
            "embeddings": embeddings[i * B : (i + 1) * B],
            "W_in1": W_in1,
            "W_out1": W_out1,
            "W_in2": W_in2,
            "W_out2": W_out2,
        }
        for i in range(N_CORES)
    ]
    res = run_bass_kernel_spmd(nc, in_maps, core_ids=list(range(N_CORES)))
    pooled = np.concatenate([res.results[i]["out"] for i in range(N_CORES)], axis=0)
    return pooled.reshape(N_FULL, D, 1, 1).astype(np.float32)


if __name__ == "__main__":
    rng = np.random.default_rng(0)
    inputs = {
        "embeddings": rng.standard_normal((N_FULL, L, D), dtype=np.float32),
        "W_in1": (rng.standard_normal((D, D)) * 0.02).astype(np.float32),
        "W_out1": (rng.standard_normal((D, C)) * 0.02).astype(np.float32),
        "W_in2": (rng.standard_normal((D, D)) * 0.02).astype(np.float32),
        "W_out2": (rng.standard_normal((D, C)) * 0.02).astype(np.float32),
    }
    out = kernel(**inputs)
    print(out.shape, out.dtype)


# revision 6
# speedup vs baseline: 1.4529x; 1.4529x over previous
"""Trainium2 Bass kernel for AttentionFuserV2 (dense transformer, 2 attention
blocks + mean-pool), data-parallel over 8 NeuronCores.

Math per batch element (x = [L, D] slice of embeddings):
  block(x, Win, Wout, tanh?):
    q = x @ Win^T; s = q @ x^T; w = softmax(s); mix = w @ x
    out = [mix, q] @ Wout^T; tanh optional
  x1 = block(x, Win1, Wout1, tanh); x2 = block(x1, Win2, Wout2)
  result = mean_l(x2)

Block-2 shortcut (mean is linear): result = (1/L) * combbar @ Wout2^T with
  combbar = [mixbar ; qbar], qbar = sum_l q2[l,:],
  mixbar[d] = sum_m c2[m] * x1[m,d], c2[m] = sum_l softmax(s2)[l,m]
so mix2/out2 ([L,D] / [L,2D] matmuls) are never materialized.

Implementation notes:
 - partition-inner chunked layouts: [128, K, F] tile holds a [128K, F] matrix
   with logical row i at (k=i//128, p=i%128).
 - all transposes are HWDGE DMA-transposes on bf16 (no PE cycles).
 - softmax skips max subtraction (score ranges are bounded; exp is computed
   in fp32 from PSUM) and gets row sums free via the ACT exp's accum_out.
 - c2 is computed as a [1,512] row via 4 matmuls with the reciprocal row-sum
   column as lhsT; mixbar = x1^T @ c2 is done on the Vector engine with a
   DMA partition-broadcast of c2 and fused multiply-reduce (the 32 tiny
   matmuls this replaces were LDWEIGHTS-bound on the PE).
 - the per-element dataflow is emitted as a 6-stage software pipeline
   (A0 load/cast/x^T | A1 q1/s1/softmax/w1^T | B mix/out1/tanh/x1^T |
   C1 q2/s2/exp2 | C2 c2/mixbar/qbar | C3 pooled matvec + store) skewed
   across batch elements so the in-order engines always have ready work —
   emitted program order is the schedule on each engine.
"""

import os
import sys

import numpy as np

if "/opt/trn_rl_repo" not in sys.path:
    sys.path.insert(0, "/opt/trn_rl_repo")

P = 128
L = 512          # sequence length
D = 512          # model dim
C = 2 * D        # concat dim
KL = L // P      # 4
KD = D // P      # 4
KC = C // P      # 8
N_FULL = 256     # full batch
N_CORES = 8
B = N_FULL // N_CORES  # 32 batch elements per core

_NC_CACHE = {}


def build_nc(b_per_core: int):
    PIPE = os.environ.get("K_PIPE", "1") == "1"
    TTR = os.environ.get("K_TTR", "0") == "1"
    ACTCOPY = os.environ.get("K_ACTCOPY", "1") == "1"
    import concourse.bass as bass  # noqa: F401
    import concourse.mybir as mybir
    from concourse import bacc
    from concourse.tile import TileContext

    FP32 = mybir.dt.float32
    BF16 = mybir.dt.bfloat16
    EXP = mybir.ActivationFunctionType.Exp
    TANH = mybir.ActivationFunctionType.Tanh
    COPY = mybir.ActivationFunctionType.Copy
    AX = mybir.AxisListType.X
    MUL = mybir.AluOpType.mult
    ADD = mybir.AluOpType.add

    nc = bacc.Bacc()

    emb = nc.declare_dram_parameter("embeddings", [b_per_core, L, D], FP32, isOutput=False)
    w_in1 = nc.declare_dram_parameter("W_in1", [D, D], FP32, isOutput=False)
    w_out1 = nc.declare_dram_parameter("W_out1", [D, C], FP32, isOutput=False)
    w_in2 = nc.declare_dram_parameter("W_in2", [D, D], FP32, isOutput=False)
    w_out2 = nc.declare_dram_parameter("W_out2", [D, C], FP32, isOutput=False)
    out = nc.declare_dram_parameter("out", [b_per_core, D], FP32, isOutput=True)

    with TileContext(nc) as tc:
        import contextlib

        ctx = contextlib.ExitStack()
        with ctx:
            wpool = ctx.enter_context(tc.tile_pool(name="wpool", bufs=1))
            xpool = ctx.enter_context(tc.tile_pool(name="xpool", bufs=2))
            apool = ctx.enter_context(tc.tile_pool(name="apool", bufs=3))
            spool = ctx.enter_context(tc.tile_pool(name="spool", bufs=4))
            dpool = ctx.enter_context(tc.tile_pool(name="dpool", bufs=3, space="DRAM"))
            psum = ctx.enter_context(tc.tile_pool(name="psum", bufs=6, space="PSUM"))

            # ---- one-time weight prep: f32 HBM -> bf16 transposed SBUF ----
            # WinT[i][p, kd, e]  = Win_i[e, kd*128+p]   (Win^T, d on partitions)
            # WoutT[i][p, kc, d] = Wout_i[d, kc*128+p]  (Wout^T, c on partitions)
            win_T = []
            wout_T = []
            with tc.tile_pool(name="wprep", bufs=1) as prep:
                for i, w_in in enumerate((w_in1, w_in2)):
                    wtmp = prep.tile([P, KD, D], FP32, name=f"wtmp_in{i}", tag="wtmp", padded_shape=[P, KD, C])
                    nc.gpsimd.dma_start(out=wtmp, in_=w_in.rearrange("(k p) d -> p k d", p=P))
                    wtmp_bf = prep.tile([P, KD, D], BF16, name=f"wtmpb_in{i}", tag="wtmpb", padded_shape=[P, KD, C])
                    nc.vector.tensor_copy(out=wtmp_bf, in_=wtmp)
                    wT = wpool.tile([P, KD, D], BF16, name=f"win_T{i}")
                    for ke in range(KD):
                        nc.sync.dma_start_transpose(
                            wT[:, :, ke * P : (ke + 1) * P], wtmp_bf[:, ke, :]
                        )
                    win_T.append(wT)
                for i, w_out in enumerate((w_out1, w_out2)):
                    wtmp2 = prep.tile([P, KD, C], FP32, name=f"wtmp_out{i}", tag="wtmp")
                    nc.gpsimd.dma_start(out=wtmp2, in_=w_out.rearrange("(k p) c -> p k c", p=P))
                    wtmp2_bf = prep.tile([P, KD, C], BF16, name=f"wtmpb_out{i}", tag="wtmpb")
                    nc.vector.tensor_copy(out=wtmp2_bf, in_=wtmp2)
                    wT = wpool.tile([P, KC, D], BF16, name=f"wout_T{i}")
                    for kd in range(KD):
                        nc.sync.dma_start_transpose(
                            wT[:, :, kd * P : (kd + 1) * P], wtmp2_bf[:, kd, :]
                        )
                    wout_T.append(wT)

            inv_L = 1.0 / float(L)

            # Cross-stage tiles, keyed by element index.
            live = {}

            def stage_a0(e):
                x_f32 = xpool.tile([P, KL, D], FP32, name="x_f32", bufs=2)
                nc.gpsimd.dma_start(out=x_f32, in_=emb[e].rearrange("(k p) d -> p k d", p=P))
                x_nat = apool.tile([P, KL, D], BF16, name="x_nat", bufs=3)
                nc.gpsimd.tensor_copy(out=x_nat, in_=x_f32)
                xT = apool.tile([P, KD, L], BF16, name="xT", bufs=3)
                for km in range(KL):
                    nc.sync.dma_start_transpose(xT[:, :, km * P : (km + 1) * P], x_nat[:, km, :])
                live[e] = {"x_nat": x_nat, "xT": xT}

            def stage_a1(e):
                st = live[e]
                xT = st["xT"]
                q1T = apool.tile([P, KD, L], BF16, name="q1T", bufs=3)
                for eh in range(KD):
                    ps_q = psum.tile([P, L], FP32, name="ps_q", tag="mm")
                    for kd in range(KD):
                        nc.tensor.matmul(
                            ps_q,
                            lhsT=win_T[0][:, kd, eh * P : (eh + 1) * P],
                            rhs=xT[:, kd, :],
                            start=(kd == 0),
                            stop=(kd == KD - 1),
                        )
                    if ACTCOPY:
                        nc.scalar.activation(out=q1T[:, eh, :], in_=ps_q, func=COPY)
                    else:
                        nc.vector.tensor_copy(out=q1T[:, eh, :], in_=ps_q)

                w1 = apool.tile([P, KL, L], BF16, name="w1", bufs=2)
                rs1 = spool.tile([P, KL], FP32, name="rs1", tag="rs1")
                for lh in range(KL):
                    ps_s = psum.tile([P, L], FP32, name="ps_s", tag="mm")
                    for ke in range(KD):
                        nc.tensor.matmul(
                            ps_s,
                            lhsT=q1T[:, ke, lh * P : (lh + 1) * P],
                            rhs=xT[:, ke, :],
                            start=(ke == 0),
                            stop=(ke == KD - 1),
                        )
                    nc.scalar.activation(
                        out=w1[:, lh, :], in_=ps_s, func=EXP, accum_out=rs1[:, lh : lh + 1]
                    )
                r1 = spool.tile([P, KL], FP32, name="r1", tag="r1")
                nc.vector.reciprocal(out=r1, in_=rs1)
                for lh in range(KL):
                    nc.vector.tensor_scalar_mul(w1[:, lh, :], w1[:, lh, :], r1[:, lh : lh + 1])
                w1T = apool.tile([P, KL, L], BF16, name="w1T", bufs=3)
                for lh in range(KL):
                    nc.sync.dma_start_transpose(w1T[:, :, lh * P : (lh + 1) * P], w1[:, lh, :])
                st.update(q1T=q1T, w1T=w1T)

            def stage_b(e):
                st = live[e]
                x_nat, q1T, w1T = st["x_nat"], st["q1T"], st["w1T"]
                mixT = apool.tile([P, KD, L], BF16, name="mixT", bufs=2)
                for dh in range(KD):
                    ps_m = psum.tile([P, L], FP32, name="ps_m", tag="mm")
                    for km in range(KL):
                        nc.tensor.matmul(
                            ps_m,
                            lhsT=x_nat[:, km, dh * P : (dh + 1) * P],
                            rhs=w1T[:, km, :],
                            start=(km == 0),
                            stop=(km == KL - 1),
                        )
                    nc.vector.tensor_copy(out=mixT[:, dh, :], in_=ps_m)

                x1_nat = apool.tile([P, KL, D], BF16, name="x1_nat", bufs=2)
                for lh in range(KL):
                    ps_o = psum.tile([P, D], FP32, name="ps_o", tag="mm")
                    for kc in range(KC):
                        lhsT_src = (
                            mixT[:, kc, lh * P : (lh + 1) * P]
                            if kc < KD
                            else q1T[:, kc - KD, lh * P : (lh + 1) * P]
                        )
                        nc.tensor.matmul(
                            ps_o,
                            lhsT=lhsT_src,
                            rhs=wout_T[0][:, kc, :],
                            start=(kc == 0),
                            stop=(kc == KC - 1),
                        )
                    nc.scalar.activation(out=x1_nat[:, lh, :], in_=ps_o, func=TANH)

                x1T = apool.tile([P, KD, L], BF16, name="x1T", bufs=3)
                for km in range(KL):
                    nc.sync.dma_start_transpose(x1T[:, :, km * P : (km + 1) * P], x1_nat[:, km, :])
                st.update(x1T=x1T)
                del st["x_nat"], st["xT"], st["q1T"], st["w1T"]

            def stage_c1(e):
                st = live[e]
                x1T = st["x1T"]
                q2T = apool.tile([P, KD, L], BF16, name="q2T", bufs=3)
                for eh in range(KD):
                    ps_q2 = psum.tile([P, L], FP32, name="ps_q2", tag="mm")
                    for kd in range(KD):
                        nc.tensor.matmul(
                            ps_q2,
                            lhsT=win_T[1][:, kd, eh * P : (eh + 1) * P],
                            rhs=x1T[:, kd, :],
                            start=(kd == 0),
                            stop=(kd == KD - 1),
                        )
                    if ACTCOPY:
                        nc.scalar.activation(out=q2T[:, eh, :], in_=ps_q2, func=COPY)
                    else:
                        nc.vector.tensor_copy(out=q2T[:, eh, :], in_=ps_q2)

                e2 = apool.tile([P, KL, L], BF16, name="e2", bufs=3)
                rs2 = spool.tile([P, KL], FP32, name="rs2", tag="rs2")
                for lh in range(KL):
                    ps_s2 = psum.tile([P, L], FP32, name="ps_s2", tag="mm")
                    for ke in range(KD):
                        nc.tensor.matmul(
                            ps_s2,
                            lhsT=q2T[:, ke, lh * P : (lh + 1) * P],
                            rhs=x1T[:, ke, :],
                            start=(ke == 0),
                            stop=(ke == KD - 1),
                        )
                    nc.scalar.activation(
                        out=e2[:, lh, :], in_=ps_s2, func=EXP, accum_out=rs2[:, lh : lh + 1]
                    )
                r2 = spool.tile([P, KL], FP32, name="r2", tag="r2")
                nc.vector.reciprocal(out=r2, in_=rs2)
                r2b = spool.tile([P, KL], BF16, name="r2b", tag="r2b")
                nc.vector.tensor_copy(out=r2b, in_=r2)
                st.update(q2T=q2T, e2=e2, r2b=r2b)

            def stage_c2(e):
                st = live[e]
                x1T, q2T, e2, r2b = st["x1T"], st["q2T"], st["e2"], st["r2b"]
                # c2 row: c2[m] = sum_l exp(s2)[l, m] / rowsum2[l]
                ps_c2 = psum.tile([1, L], FP32, name="ps_c2", tag="row", bufs=2)
                for kl in range(KL):
                    nc.tensor.matmul(
                        ps_c2,
                        lhsT=r2b[:, kl : kl + 1],
                        rhs=e2[:, kl, :],
                        start=(kl == 0),
                        stop=(kl == KL - 1),
                    )
                c2row = spool.tile([1, L], FP32, name="c2row", tag="c2row")
                nc.vector.tensor_copy(out=c2row, in_=ps_c2)
                # partition-broadcast c2 via DRAM roundtrip
                c2d = dpool.tile([1, L], FP32, name="c2d")
                nc.gpsimd.dma_start(out=c2d, in_=c2row)
                c2bc = apool.tile([P, L], FP32, name="c2bc", bufs=2)
                nc.gpsimd.dma_start(out=c2bc, in_=c2d.to_broadcast((P, L)))

                qmb = spool.tile([P, KC], FP32, name="qmb", tag="qmb")
                # qbar first (no DMA dependency), then mixbar via fused mul+reduce
                for ke in range(KD):
                    nc.vector.reduce_sum(
                        out=qmb[:, KD + ke : KD + ke + 1], in_=q2T[:, ke, :], axis=AX
                    )
                scr = apool.tile([P, L], BF16, name="scr", bufs=2)
                if TTR:
                    for kd in range(KD):
                        nc.vector.tensor_tensor_reduce(
                            out=scr,
                            in0=x1T[:, kd, :],
                            in1=c2bc,
                            scale=1.0,
                            scalar=0.0,
                            op0=MUL,
                            op1=ADD,
                            accum_out=qmb[:, kd : kd + 1],
                        )
                else:
                    for kd in range(KD):
                        nc.vector.tensor_mul(out=scr, in0=x1T[:, kd, :], in1=c2bc)
                        nc.vector.reduce_sum(out=qmb[:, kd : kd + 1], in_=scr, axis=AX)
                cb = spool.tile([P, KC], BF16, name="cb", tag="cb")
                nc.vector.tensor_copy(out=cb, in_=qmb)
                st.update(cb=cb)
                del st["x1T"], st["q2T"], st["e2"], st["r2b"]

            def stage_c3(e):
                st = live.pop(e)
                cb = st["cb"]
                ps_p = psum.tile([1, D], FP32, name="ps_p", tag="row", bufs=2)
                for kc in range(KC):
                    nc.tensor.matmul(
                        ps_p,
                        lhsT=cb[:, kc : kc + 1],
                        rhs=wout_T[1][:, kc, :],
                        start=(kc == 0),
                        stop=(kc == KC - 1),
                    )
                out_row = spool.tile([1, D], FP32, name="out_row", tag="out_row")
                nc.scalar.activation(out=out_row, in_=ps_p, func=COPY, scale=inv_L)
                nc.gpsimd.dma_start(out=out[e : e + 1, :], in_=out_row)

            stages = [stage_a0, stage_a1, stage_b, stage_c1, stage_c2, stage_c3]
            n_st = len(stages)
            if PIPE:
                for t in range(b_per_core + n_st - 1):
                    for si, fn in enumerate(stages):
                        e = t - si
                        if 0 <= e < b_per_core:
                            fn(e)
            else:
                for e in range(b_per_core):
                    for fn in stages:
                        fn(e)

    nc.finalize()
    return nc


def _get_nc(b_per_core: int):
    if b_per_core not in _NC_CACHE:
        _NC_CACHE[b_per_core] = build_nc(b_per_core)
    return _NC_CACHE[b_per_core]


def kernel(embeddings, W_in1, W_out1, W_in2, W_out2):
    from concourse.bass_utils import run_bass_kernel_spmd

    embeddings = np.ascontiguousarray(np.asarray(embeddings, dtype=np.float32))
    W_in1 = np.ascontiguousarray(np.asarray(W_in1, dtype=np.float32))
    W_out1 = np.ascontiguousarray(np.asarray(W_out1, dtype=np.float32))
    W_in2 = np.ascontiguousarray(np.asarray(W_in2, dtype=np.float32))
    W_out2 = np.ascontiguousarray(np.asarray(W_out2, dtype=np.float32))

    assert embeddings.shape == (N_FULL, L, D), embeddings.shape

    nc = _get_nc(B)
    in_maps = [
        {
            "embeddings": embeddings[i * B : (i + 1) * B],
            "W_in1": W_in1,
            "W_out1": W_out1,
            "W_in2": W_in2,
            "W_out2": W_out2,
        }
        for i in range(N_CORES)
    ]
    res = run_bass_kernel_spmd(nc, in_maps, core_ids=list(range(N_CORES)))
    pooled = np.concatenate([res.results[i]["out"] for i in range(N_CORES)], axis=0)
    return pooled.reshape(N_FULL, D, 1, 1).astype(np.float32)


if __name__ == "__main__":
    rng = np.random.default_rng(0)
    inputs = {
        "embeddings": rng.standard_normal((N_FULL, L, D), dtype=np.float32),
        "W_in1": (rng.standard_normal((D, D)) * 0.02).astype(np.float32),
        "W_out1": (rng.standard_normal((D, C)) * 0.02).astype(np.float32),
        "W_in2": (rng.standard_normal((D, D)) * 0.02).astype(np.float32),
        "W_out2": (rng.standard_normal((D, C)) * 0.02).astype(np.float32),
    }
    out = kernel(**inputs)
    print(out.shape, out.dtype)
